# revision 1
# baseline (speedup 1.0000x reference)
"""Trainium2 Bass kernel for nn_ClassifyMLPHeadForKCRWithConcatChoices.

Math (B=16, L=2048, H=A=1024, C=5):
  keys  = tanh(X @ Wh^T + bh)                    (B,L,A)
  probs = keys @ (q / sqrt(A*var(q)))            (B,L)
  z     = probs * (-1000 * (1 - attn))           (B,L)
  att   = softmax_L(z)                           (B,L)
  vals  = att[...,None] + X                      (B,L,H)
  ctx   = einsum('bcl,blh->bch', seg, vals)
  logit = ctx @ Wc^T + bc                        (B,C,1)

Because att broadcasts over H and the classifier is rank-1:
  logit[b,c] = (seg·att)[b,c] * sum(Wc) + (seg·y)[b,c] + bc,  y = X @ Wc
so the device only computes the heavy parts — keys/probs (68.7 GFLOP matmul +
tanh), the per-row softmax, and the per-token classifier projection y — and
returns per-token att and y.  The O(B*C*L) segment pooling runs on the host
during unsharding.

Sharding: data-parallel over batch, 2 rows per core x 8 cores; weights
replicated.  X is pre-transposed on the host to (H, tokens) so the contraction
dim lies on SBUF partitions, and cast to bf16 (PE fp32 matmul is 4.5x slower;
validated end-to-end rel err ~2e-3).
"""

import sys

if '/opt/trn_rl_repo' not in sys.path:
    sys.path.insert(0, '/opt/trn_rl_repo')

import numpy as np
import ml_dtypes

import concourse.bass as bass  # noqa: F401  (bass must import before bacc)
import concourse.mybir as mybir
import concourse.tile as tile
from concourse import bacc
from concourse.bass_utils import run_bass_kernel_spmd

B, L, H, A, C = 16, 2048, 1024, 1024, 5
N_CORES = 8
RPC = B // N_CORES          # batch rows per core
NTOK = RPC * L              # tokens per core
P = 128
HB, AB = H // P, A // P     # contraction / output blocks
CH = 512                    # token chunk (one PSUM bank)
NCH = NTOK // CH

BF16 = mybir.dt.bfloat16
FP32 = mybir.dt.float32
FP8 = mybir.dt.float8e4
NP_FP8 = mybir.dt.np(FP8)
MODE = "fp8"  # "fp8" (DoubleRow keys matmul) or "bf16"


def build_program(repeat: int = 1, n_cores: int = N_CORES,
                  tail: str = "online", mode: str = MODE,
                  bias_free: bool = True, pk_bufs: int = 2, ps_bufs: int = 2):
    """mode="fp8": keys matmul runs fp8e4 with DoubleRow (2 h-blocks per MM),
    tanh is emitted fp8 and merged across a-block pairs (when b_hidden == 0,
    bias_free=True), and the probs matvec contracts 2 a-blocks per DoubleRow
    MM; the classifier projection y stays bf16 (its precision reaches the
    output; keys precision is absorbed by the softmax's huge mask margin)."""
    nc = bacc.Bacc("TRN2", target_bir_lowering=False, debug=False,
                   num_devices=n_cores)
    xt_d = nc.dram_tensor("xt", [HB, P, NTOK], BF16, kind="ExternalInput")
    if mode == "fp8":
        # q padded to 16B per a-block: dual-fp8 LDWEIGHTS requires the
        # weight AP's block step to be a multiple of 16 bytes
        qs_d = nc.dram_tensor("qs", [P, AB * 16], FP8, kind="ExternalInput")
    else:
        qs_d = nc.dram_tensor("qs", [P, AB], BF16, kind="ExternalInput")
    wc_d = nc.dram_tensor("wc", [P, HB], BF16, kind="ExternalInput")
    bh_d = nc.dram_tensor("bh", [P, AB], FP32, kind="ExternalInput")
    mm_d = nc.dram_tensor("mm", [1, NTOK], FP32, kind="ExternalInput")
    if mode == "fp8":
        xt8_d = nc.dram_tensor("xt8", [NCH, P, HB * CH], FP8,
                               kind="ExternalInput")
        wht8_d = nc.dram_tensor("wht8", [P, HB * A], FP8, kind="ExternalInput")
    else:
        wht_d = nc.dram_tensor("wht", [HB, P, A], BF16, kind="ExternalInput")
    out_d = nc.dram_tensor("out", [2, NTOK], FP32, kind="ExternalOutput")

    with tile.TileContext(nc) as tc:
        with (
            tc.tile_pool(name="const", bufs=1) as const,
            tc.tile_pool(name="xpool", bufs=1) as xpool,
            tc.tile_pool(name="keys", bufs=3) as keys,
            tc.tile_pool(name="vecs", bufs=1) as vecs,
            tc.tile_pool(name="ps_k", bufs=pk_bufs, space="PSUM") as ps_k,
            tc.tile_pool(name="ps_s", bufs=ps_bufs, space="PSUM") as ps_s,
        ):
            if mode == "fp8":
                wht8_sb = const.tile([P, HB, A], FP8)
                nc.sync.dma_start(
                    wht8_sb[:],
                    wht8_d.ap().rearrange("p (h a) -> p h a", h=HB))
            else:
                wht_sb = const.tile([P, HB, A], BF16)
                for hb in range(HB):
                    nc.sync.dma_start(wht_sb[:, hb, :], wht_d.ap()[hb])
            if mode == "fp8":
                qs_sb = const.tile([P, AB, 16], FP8)
                nc.sync.dma_start(
                    qs_sb[:], qs_d.ap().rearrange("p (a s) -> p a s", a=AB))
            else:
                qs_sb = const.tile([P, AB], BF16)
                nc.sync.dma_start(qs_sb[:], qs_d.ap())
            wc_sb = const.tile([P, HB], BF16)
            nc.sync.dma_start(wc_sb[:], wc_d.ap())
            bh_sb = const.tile([P, AB], FP32)
            nc.sync.dma_start(bh_sb[:], bh_d.ap())
            mm_sb = const.tile([1, NTOK], FP32)
            nc.sync.dma_start(mm_sb[:], mm_d.ap())
            # prefetch the exp_and_others ACT table set (covers Tanh + Exp)
            # during the input DMA window instead of at first real tanh
            warm = const.tile([1, 1], FP32)
            nc.scalar.activation(warm[:], mm_sb[:, 0:1],
                                 mybir.ActivationFunctionType.Tanh)

            # X^T staged per (hb, chunk) so compute can start after the first
            # column of h-blocks lands.
            xt_sb = {}
            xt8_sb = {}
            for ch in range(NCH):
                if mode == "fp8":
                    t8 = xpool.tile([P, HB, CH], FP8, tag=f"x8_{ch}")
                    nc.sync.dma_start(
                        t8[:],
                        xt8_d.ap()[ch].rearrange("p (h t) -> p h t", h=HB))
                    xt8_sb[ch] = t8
                for hb in range(HB):
                    t = xpool.tile([P, CH], BF16, tag=f"x{hb}_{ch}")
                    nc.sync.dma_start(
                        t[:], xt_d.ap()[hb, :, ch * CH:(ch + 1) * CH])
                    xt_sb[hb, ch] = t

            CPR = NCH // RPC  # chunks per batch row
            for _ in range(repeat):
                y_sb = vecs.tile([1, NTOK], FP32, tag="y")
                z_sb = vecs.tile([1, NTOK], FP32, tag="z")
                e_sb = vecs.tile([1, NTOK], FP32, tag="e")
                ncmax_sb = vecs.tile([1, NCH], FP32, tag="ncmax")
                csum_sb = vecs.tile([1, NCH], FP32, tag="csum")
                att_sb = vecs.tile([1, NTOK], FP32, tag="att")
                for ch in range(NCH):
                    sl = slice(ch * CH, (ch + 1) * CH)
                    chsl = slice(ch, ch + 1)
                    pprobs = ps_s.tile([1, CH], FP32, tag="pprobs")
                    if mode == "fp8":
                        for abp in range(AB // 2):
                            pk2 = ps_k.tile([P, 2, CH], FP32, tag="pk2")
                            for j in range(2):
                                ab = 2 * abp + j
                                for hbp in range(HB // 2):
                                    nc.tensor.matmul(
                                        pk2[:, j, :],
                                        lhsT=wht8_sb[:, 2 * hbp:2 * hbp + 2,
                                                     ab * P:(ab + 1) * P],
                                        rhs=xt8_sb[ch][:, 2 * hbp:2 * hbp + 2, :],
                                        start=(hbp == 0),
                                        stop=(hbp == HB // 2 - 1),
                                        perf_mode=mybir.MatmulPerfMode.DoubleRow,
                                    )
                            ks2 = keys.tile([P, 2, CH], FP8, tag="ks2")
                            if bias_free:
                                nc.scalar.activation(
                                    ks2[:], pk2[:],
                                    mybir.ActivationFunctionType.Tanh)
                            else:
                                for j in range(2):
                                    nc.scalar.activation(
                                        ks2[:, j, :], pk2[:, j, :],
                                        mybir.ActivationFunctionType.Tanh,
                                        bias=bh_sb[:, 2 * abp + j:
                                                   2 * abp + j + 1], scale=1.0)
                            nc.tensor.matmul(
                                pprobs[:],
                                lhsT=qs_sb[:, 2 * abp:2 * abp + 2, 0:1],
                                rhs=ks2[:],
                                start=(abp == 0), stop=(abp == AB // 2 - 1),
                                perf_mode=mybir.MatmulPerfMode.DoubleRow)
                    else:
                        for ab in range(AB):
                            pk = ps_k.tile([P, CH], FP32, tag="pk")
                            for hb in range(HB):
                                nc.tensor.matmul(
                                    pk[:],
                                    lhsT=wht_sb[:, hb, ab * P:(ab + 1) * P],
                                    rhs=xt_sb[hb, ch][:],
                                    start=(hb == 0), stop=(hb == HB - 1),
                                )
                            ks = keys.tile([P, CH], BF16, tag="ks")
                            nc.scalar.activation(
                                ks[:], pk[:],
                                mybir.ActivationFunctionType.Tanh,
                                bias=bh_sb[:, ab:ab + 1], scale=1.0)
                            nc.tensor.matmul(
                                pprobs[:], lhsT=qs_sb[:, ab:ab + 1], rhs=ks[:],
                                start=(ab == 0), stop=(ab == AB - 1))
                    py = ps_s.tile([1, CH], FP32, tag="py")
                    for hb in range(HB):
                        nc.tensor.matmul(
                            py[:], lhsT=wc_sb[:, hb:hb + 1],
                            rhs=xt_sb[hb, ch][:],
                            start=(hb == 0), stop=(hb == HB - 1))
                    nc.vector.tensor_copy(y_sb[:, sl], py[:])
                    # z = probs * maskmul, fused from PSUM; per-chunk -max
                    nc.vector.tensor_mul(z_sb[:, sl], pprobs[:], mm_sb[:, sl])
                    nc.vector.reduce_max(ncmax_sb[:, chsl], z_sb[:, sl],
                                         axis=mybir.AxisListType.X, negate=True)
                    if tail == "online":
                        nc.scalar.activation(
                            e_sb[:, sl], z_sb[:, sl],
                            mybir.ActivationFunctionType.Exp,
                            bias=ncmax_sb[:, chsl], scale=1.0)
                        nc.vector.reduce_sum(csum_sb[:, chsl], e_sb[:, sl],
                                             axis=mybir.AxisListType.X)

                if tail == "online":
                    # combine chunks per batch row: with M_r = max_ch cmax_ch,
                    # f_ch = exp(cmax_ch - M_r), Z_r = sum_ch csum_ch * f_ch,
                    # att = e_ch * f_ch / Z_r
                    for r in range(RPC):
                        rsl = slice(r * CPR, (r + 1) * CPR)
                        nmax = vecs.tile([1, 1], FP32, tag=f"nmax{r}")
                        # ncmax holds -cmax; nmax := -M_r = min(ncmax)
                        nc.vector.tensor_reduce(nmax[:], ncmax_sb[:, rsl],
                                                axis=mybir.AxisListType.X,
                                                op=mybir.AluOpType.min)
                        # f_ch = exp(cmax_ch - M_r) = Exp(-1 * ncmax_ch + nmax)
                        f_sb = vecs.tile([1, CPR], FP32, tag=f"f{r}")
                        nc.scalar.activation(
                            f_sb[:], ncmax_sb[:, rsl],
                            mybir.ActivationFunctionType.Exp,
                            bias=nmax[:], scale=-1.0)
                        zr = vecs.tile([1, CPR], FP32, tag=f"zr{r}")
                        nc.vector.tensor_mul(zr[:], csum_sb[:, rsl], f_sb[:])
                        zsum = vecs.tile([1, 1], FP32, tag=f"zsum{r}")
                        nc.vector.reduce_sum(zsum[:], zr[:],
                                             axis=mybir.AxisListType.X)
                        rz = vecs.tile([1, 1], FP32, tag=f"rz{r}")
                        nc.vector.reciprocal(rz[:], zsum[:])
                        g_sb = vecs.tile([1, CPR], FP32, tag=f"g{r}")
                        nc.vector.tensor_scalar_mul(g_sb[:], f_sb[:],
                                                    scalar1=rz[:])
                        for k in range(CPR):
                            ch = r * CPR + k
                            sl = slice(ch * CH, (ch + 1) * CH)
                            nc.vector.tensor_scalar_mul(
                                att_sb[:, sl], e_sb[:, sl],
                                scalar1=g_sb[:, k:k + 1])
                        rowsl = slice(r * L, (r + 1) * L)
                        nc.sync.dma_start(out_d.ap()[0:1, rowsl],
                                          att_sb[:, rowsl])
                else:
                    # simple tail: one exp/sum/scale per batch row
                    for r in range(RPC):
                        rowsl = slice(r * L, (r + 1) * L)
                        rsl = slice(r * CPR, (r + 1) * CPR)
                        nmax = vecs.tile([1, 1], FP32, tag=f"nmax{r}")
                        nc.vector.tensor_reduce(nmax[:], ncmax_sb[:, rsl],
                                                axis=mybir.AxisListType.X,
                                                op=mybir.AluOpType.min)
                        nc.scalar.activation(
                            e_sb[:, rowsl], z_sb[:, rowsl],
                            mybir.ActivationFunctionType.Exp,
                            bias=nmax[:], scale=1.0)
                        zsum = vecs.tile([1, 1], FP32, tag=f"zsum{r}")
                        nc.vector.reduce_sum(zsum[:], e_sb[:, rowsl],
                                             axis=mybir.AxisListType.X)
                        rz = vecs.tile([1, 1], FP32, tag=f"rz{r}")
                        nc.vector.reciprocal(rz[:], zsum[:])
                        nc.vector.tensor_scalar_mul(att_sb[:, rowsl],
                                                    e_sb[:, rowsl],
                                                    scalar1=rz[:])
                        rowsl2 = slice(r * L, (r + 1) * L)
                        nc.sync.dma_start(out_d.ap()[0:1, rowsl2],
                                          att_sb[:, rowsl2])
                nc.sync.dma_start(out_d.ap()[1:2, :], y_sb[:])

    nc.compile()
    return nc


def prep_inputs(inputs):
    """Full inputs -> (per-core in_maps, host epilogue constants)."""
    X = np.ascontiguousarray(np.asarray(inputs["input"], dtype=np.float32))
    attn = np.asarray(inputs["attention_mask"])
    mlm = np.asarray(inputs["mlm_mask"])
    Wh = np.asarray(inputs["W_hidden"], dtype=np.float32)
    bh = np.asarray(inputs["b_hidden"], dtype=np.float32)
    q = np.asarray(inputs["query"], dtype=np.float32)[:, 0]
    Wc = np.asarray(inputs["W_cls"], dtype=np.float32)[0]
    bc = float(np.asarray(inputs["b_cls"], dtype=np.float32)[0])

    qvar = np.var(q.astype(np.float64), ddof=1)
    scale = 1.0 / np.sqrt(A * qvar)

    WhT = np.ascontiguousarray(Wh.T)  # (H, A)
    if MODE == "fp8":
        qs = np.zeros((P, AB, 16), NP_FP8)
        qs[:, :, 0] = (q * scale).reshape(AB, P).T.astype(NP_FP8)
        qs = qs.reshape(P, AB * 16)
    else:
        qs = np.ascontiguousarray(
            (q * scale).reshape(AB, P).T).astype(ml_dtypes.bfloat16)
    wc = np.ascontiguousarray(Wc.reshape(HB, P).T).astype(ml_dtypes.bfloat16)
    bh_a = np.ascontiguousarray(bh.reshape(AB, P).T).astype(np.float32)
    maskmul = ((1.0 - attn.astype(np.float32)) * -1000.0)
    if MODE == "fp8":
        # wht8[p, hb*A + a] = WhT[hb*128+p, a]
        wht8 = np.ascontiguousarray(
            WhT.reshape(HB, P, A).transpose(1, 0, 2).reshape(P, HB * A)
        ).astype(NP_FP8)
    else:
        wht = WhT.reshape(HB, P, A).astype(ml_dtypes.bfloat16)

    XT = X.reshape(B * L, H).T  # (H, B*L) view
    in_maps = []
    for c in range(N_CORES):
        xt_c = np.ascontiguousarray(
            XT[:, c * NTOK:(c + 1) * NTOK]).reshape(HB, P, NTOK)
        m = dict(
            xt=xt_c.astype(ml_dtypes.bfloat16),
            qs=qs, wc=wc, bh=bh_a,
            mm=np.ascontiguousarray(
                maskmul.reshape(1, B * L)[:, c * NTOK:(c + 1) * NTOK]),
        )
        if MODE == "fp8":
            # xt8[ch, p, hb*CH + t] = XT_core[hb*128+p, ch*CH + t]
            m["xt8"] = np.ascontiguousarray(
                xt_c.reshape(HB, P, NCH, CH).transpose(2, 1, 0, 3)
                .reshape(NCH, P, HB * CH)).astype(NP_FP8)
            m["wht8"] = wht8
        else:
            m["wht"] = wht
        in_maps.append(m)
    return in_maps, (attn, mlm, Wc, bc)


def epilogue(att, y, attn, mlm, Wc, bc):
    """Segment pooling + rank-1 classifier on host.  att/y: (B, L) fp32."""
    idx = np.arange(L)
    marker = np.where(mlm > 0, idx[None, :], L)
    starts = np.sort(marker, axis=1)[:, :C]
    end_idx = attn.sum(axis=1)
    bounds = np.concatenate([starts[:, 1:] - 1, (end_idx - 1)[:, None]], axis=1)
    seg = ((idx[None, None, :] >= starts[:, :, None] + 1)
           & (idx[None, None, :] < bounds[:, :, None])).astype(np.float32)
    S_att = np.einsum("bcl,bl->bc", seg, att)
    Sy = np.einsum("bcl,bl->bc", seg, y)
    Wsum = Wc.sum(dtype=np.float32)
    return (S_att * Wsum + Sy + bc).astype(np.float32)[:, :, None]


_prog_cache = {}


def kernel(**inputs) -> np.ndarray:
    bias_free = not np.any(np.asarray(inputs["b_hidden"]))
    key = ("prog", bias_free)
    if key not in _prog_cache:
        _prog_cache[key] = build_program(bias_free=bias_free)
    nc = _prog_cache[key]
    in_maps, (attn, mlm, Wc, bc) = prep_inputs(inputs)
    res = run_bass_kernel_spmd(nc, in_maps, core_ids=list(range(N_CORES)))
    att = np.concatenate(
        [res.results[c]["out"][0].reshape(RPC, L) for c in range(N_CORES)])
    y = np.concatenate(
        [res.results[c]["out"][1].reshape(RPC, L) for c in range(N_CORES)])
    return epilogue(att, y, attn, mlm, Wc, bc)



# revision 4
# speedup vs baseline: 1.5580x; 1.5580x over previous
"""Trainium2 Bass kernel for nn_ClassifyMLPHeadForKCRWithConcatChoices.

Math (B=16, L=2048, H=A=1024, C=5):
  keys  = tanh(X @ Wh^T + bh)                    (B,L,A)
  probs = keys @ (q / sqrt(A*var(q)))            (B,L)
  z     = probs * (-1000 * (1 - attn))           (B,L)
  att   = softmax_L(z)                           (B,L)
  vals  = att[...,None] + X                      (B,L,H)
  ctx   = einsum('bcl,blh->bch', seg, vals)
  logit = ctx @ Wc^T + bc                        (B,C,1)

Two structural facts make most of the FLOPs removable:

1. The softmax logits are ``probs * mask`` with mask = -1000*(1-attn), so
   z == 0 exactly wherever attn == 1.  probs (and hence the 68.7-GFLOP keys
   matmul) is only needed at PADDED tokens -- <= 511 per row, ~4.1K of 32.8K
   tokens total.  The device computes keys/probs only for a compacted,
   load-balanced gather of the padded tokens (fp8 DoubleRow matmul).
2. att broadcasts over H and the classifier is rank-1:
     logit[b,c] = (seg.att)[b,c]*sum(Wc) + (seg.y)[b,c] + bc,   y = X @ Wc
   so besides compact probs the device only needs the per-token projection
   y = X@Wc (bf16; its precision reaches the output).  y's matmul is rank-1
   (1 of 128 output partitions), so it is issued as 4 concurrent col-tiled
   matmuls (tile_position=(0,32j)) -- ~4x fewer PE cycles than sequential.

The tiny remainder (softmax over B*L scalars, segment pooling, any padded
tokens beyond device capacity) runs on the host during unsharding.

Sharding: y is data-parallel over batch (2 rows/core); compact padded tokens
are split evenly across the 8 cores regardless of row (probs is per-token).
"""

import sys

if '/opt/trn_rl_repo' not in sys.path:
    sys.path.insert(0, '/opt/trn_rl_repo')

import numpy as np
import ml_dtypes

import concourse.bass as bass  # noqa: F401  (bass must import before bacc)
import concourse.mybir as mybir
import concourse.tile as tile
from concourse import bacc
from concourse.bass_utils import run_bass_kernel_spmd

B, L, H, A, C = 16, 2048, 1024, 1024, 5
N_CORES = 8
RPC = B // N_CORES          # batch rows per core
NTOK = RPC * L              # tokens per core (y path)
P = 128
HB, AB = H // P, A // P     # contraction / output blocks
CH = 512                    # token chunk (one PSUM bank)
NCH = NTOK // CH
KCAP = 512                  # compact padded-token capacity per core (default)
LEFTOVER_BUDGET = 256       # padded tokens beyond capacity handled on host

BF16 = mybir.dt.bfloat16
FP32 = mybir.dt.float32
FP8 = mybir.dt.float8e4
NP_FP8 = mybir.dt.np(FP8)


def build_program(repeat: int = 1, n_cores: int = N_CORES,
                  kcap: int = KCAP, bias_free: bool = True,
                  pk_bufs: int = 2):
    """Compact keys/probs (fp8 DoubleRow over kcap gathered padded tokens)
    + col-tiled rank-1 classifier projection y over all NTOK tokens."""
    assert kcap % CH == 0
    kch = kcap // CH
    nc = bacc.Bacc("TRN2", target_bir_lowering=False, debug=False,
                   num_devices=n_cores)
    xt_d = nc.dram_tensor("xt", [HB, P, NTOK], BF16, kind="ExternalInput")
    xc8_d = nc.dram_tensor("xc8", [P, HB * kcap], FP8, kind="ExternalInput")
    wht8_d = nc.dram_tensor("wht8", [P, HB * A], FP8, kind="ExternalInput")
    # q padded to 16B per a-block: dual-fp8 LDWEIGHTS requires the weight
    # AP's block step to be a multiple of 16 bytes
    qs_d = nc.dram_tensor("qs", [P, AB * 16], FP8, kind="ExternalInput")
    wc_d = nc.dram_tensor("wc", [P, HB], BF16, kind="ExternalInput")
    bh_d = nc.dram_tensor("bh", [P, AB], FP32, kind="ExternalInput")
    pp_d = nc.dram_tensor("pp", [1, kcap], FP32, kind="ExternalOutput")
    y4_d = nc.dram_tensor("y4", [4, NTOK], FP32, kind="ExternalOutput")

    with tile.TileContext(nc) as tc:
        with (
            tc.tile_pool(name="const", bufs=1) as const,
            tc.tile_pool(name="xpool", bufs=1) as xpool,
            tc.tile_pool(name="keys", bufs=3) as keys,
            tc.tile_pool(name="vecs", bufs=1) as vecs,
            tc.tile_pool(name="ps_k", bufs=pk_bufs, space="PSUM") as ps_k,
            tc.tile_pool(name="ps_s", bufs=2, space="PSUM") as ps_s,
            tc.tile_pool(name="ps_y", bufs=2, space="PSUM") as ps_y,
        ):
            wht8_sb = const.tile([P, HB, A], FP8)
            nc.sync.dma_start(
                wht8_sb[:], wht8_d.ap().rearrange("p (h a) -> p h a", h=HB))
            qs_sb = const.tile([P, AB, 16], FP8)
            nc.sync.dma_start(
                qs_sb[:], qs_d.ap().rearrange("p (a s) -> p a s", a=AB))
            wc_sb = const.tile([P, HB], BF16)
            nc.sync.dma_start(wc_sb[:], wc_d.ap())
            bh_sb = const.tile([P, AB], FP32)
            nc.sync.dma_start(bh_sb[:], bh_d.ap())
            xc8_sb = const.tile([P, HB, kcap], FP8)
            nc.sync.dma_start(
                xc8_sb[:], xc8_d.ap().rearrange("p (h t) -> p h t", h=HB))
            # prefetch the exp_and_others ACT table set (covers Tanh)
            # during the input DMA window instead of at first real tanh
            warm = const.tile([1, 1], FP32)
            nc.scalar.activation(warm[:], bh_sb[:1, 0:1],
                                 mybir.ActivationFunctionType.Tanh)

            # X^T staged per (hb, chunk) for the y projection
            xt_sb = {}
            for ch in range(NCH):
                for hb in range(HB):
                    t = xpool.tile([P, CH], BF16, tag=f"x{hb}_{ch}")
                    nc.sync.dma_start(
                        t[:], xt_d.ap()[hb, :, ch * CH:(ch + 1) * CH])
                    xt_sb[hb, ch] = t

            for _ in range(repeat):
                pp_sb = vecs.tile([1, kcap], FP32, tag="pp")
                # y partials staged [128, NCH, CH]; only partitions
                # {0,32,64,96} are meaningful (col-tile outputs) -- the
                # final DMA gathers them with a partition-strided AP
                ys_sb = vecs.tile([P, NCH, CH], FP32, tag="ys")
                # --- compact keys + probs over gathered padded tokens ---
                for kc in range(kch):
                    csl = slice(kc * CH, (kc + 1) * CH)
                    pprobs = ps_s.tile([1, CH], FP32, tag="pprobs")
                    for abp in range(AB // 2):
                        pk2 = ps_k.tile([P, 2, CH], FP32, tag="pk2")
                        for j in range(2):
                            ab = 2 * abp + j
                            for hbp in range(HB // 2):
                                nc.tensor.matmul(
                                    pk2[:, j, :],
                                    lhsT=wht8_sb[:, 2 * hbp:2 * hbp + 2,
                                                 ab * P:(ab + 1) * P],
                                    rhs=xc8_sb[:, 2 * hbp:2 * hbp + 2, csl],
                                    start=(hbp == 0),
                                    stop=(hbp == HB // 2 - 1),
                                    perf_mode=mybir.MatmulPerfMode.DoubleRow,
                                )
                        ks2 = keys.tile([P, 2, CH], FP8, tag="ks2")
                        if bias_free:
                            nc.scalar.activation(
                                ks2[:], pk2[:],
                                mybir.ActivationFunctionType.Tanh)
                        else:
                            for j in range(2):
                                nc.scalar.activation(
                                    ks2[:, j, :], pk2[:, j, :],
                                    mybir.ActivationFunctionType.Tanh,
                                    bias=bh_sb[:, 2 * abp + j:
                                               2 * abp + j + 1], scale=1.0)
                        nc.tensor.matmul(
                            pprobs[:],
                            lhsT=qs_sb[:, 2 * abp:2 * abp + 2, 0:1],
                            rhs=ks2[:],
                            start=(abp == 0), stop=(abp == AB // 2 - 1),
                            perf_mode=mybir.MatmulPerfMode.DoubleRow)
                    nc.vector.tensor_copy(pp_sb[:, csl], pprobs[:])
                # --- y = X @ Wc, 4 concurrent col-tiled rank-1 matmuls ---
                for ch in range(NCH):
                    py = ps_y.tile([P, CH], FP32, tag="py")
                    for r in range(2):
                        for j in range(4):
                            hb = 4 * r + j
                            nc.tensor.matmul(
                                py[32 * j:32 * j + 1, :],
                                lhsT=wc_sb[:, hb:hb + 1],
                                rhs=xt_sb[hb, ch][:],
                                start=(r == 0), stop=(r == 1),
                                tile_position=(0, 32 * j),
                            )
                    nc.vector.tensor_copy(ys_sb[:, ch, :], py[:])
                nc.sync.dma_start(pp_d.ap()[:], pp_sb[:])
                nc.sync.dma_start(
                    y4_d.ap().rearrange("f (c t) -> f c t", c=NCH),
                    ys_sb[0:97:32, :, :])

    nc.compile()
    return nc


def prep_inputs(inputs):
    """Full inputs -> (per-core in_maps, host epilogue context)."""
    X = np.ascontiguousarray(np.asarray(inputs["input"], dtype=np.float32))
    attn = np.asarray(inputs["attention_mask"])
    mlm = np.asarray(inputs["mlm_mask"])
    Wh = np.asarray(inputs["W_hidden"], dtype=np.float32)
    bh = np.asarray(inputs["b_hidden"], dtype=np.float32)
    q = np.asarray(inputs["query"], dtype=np.float32)[:, 0]
    Wc = np.asarray(inputs["W_cls"], dtype=np.float32)[0]
    bc = float(np.asarray(inputs["b_cls"], dtype=np.float32)[0])

    qvar = np.var(q.astype(np.float64), ddof=1)
    scale = 1.0 / np.sqrt(A * qvar)

    qs = np.zeros((P, AB, 16), NP_FP8)
    qs[:, :, 0] = (q * scale).reshape(AB, P).T.astype(NP_FP8)
    qs = qs.reshape(P, AB * 16)
    wc = np.ascontiguousarray(Wc.reshape(HB, P).T).astype(ml_dtypes.bfloat16)
    bh_a = np.ascontiguousarray(bh.reshape(AB, P).T).astype(np.float32)
    WhT = np.ascontiguousarray(Wh.T)  # (H, A)
    # wht8[p, hb*A + a] = WhT[hb*128+p, a]
    wht8 = np.ascontiguousarray(
        WhT.reshape(HB, P, A).transpose(1, 0, 2).reshape(P, HB * A)
    ).astype(NP_FP8)

    # --- compact gather of padded tokens, load-balanced across cores ---
    pad_b, pad_t = np.nonzero(attn == 0)
    t_pad = len(pad_b)
    kcap = CH * max(1, int(np.ceil(
        max(t_pad - LEFTOVER_BUDGET, 1) / (N_CORES * CH))))
    n_dev = min(t_pad, N_CORES * kcap)
    flat_idx = (pad_b * L + pad_t)[:n_dev]
    # pad the slot array with token 0 (its probs output is ignored)
    slots = np.zeros(N_CORES * kcap, np.int64)
    slots[:n_dev] = flat_idx

    Xf = X.reshape(B * L, H)
    XT = Xf.T  # (H, B*L) view
    in_maps = []
    for c in range(N_CORES):
        xt_c = np.ascontiguousarray(
            XT[:, c * NTOK:(c + 1) * NTOK]).reshape(HB, P, NTOK)
        cols = slots[c * kcap:(c + 1) * kcap]
        # xc8[p, hb*kcap + t] = X[cols[t], hb*128+p]
        xc = Xf[cols].T.reshape(HB, P, kcap)  # (hb, p, t)
        xc8 = np.ascontiguousarray(
            xc.transpose(1, 0, 2).reshape(P, HB * kcap)).astype(NP_FP8)
        m = dict(
            xt=xt_c.astype(ml_dtypes.bfloat16),
            xc8=xc8, wht8=wht8, qs=qs, wc=wc, bh=bh_a,
        )
        in_maps.append(m)
    host_ctx = dict(attn=attn, mlm=mlm, Wc=Wc, bc=bc, scale=scale,
                    Wh=Wh, bh=bh, q=q, X=X, kcap=kcap, n_dev=n_dev,
                    pad_b=pad_b, pad_t=pad_t)
    return in_maps, host_ctx


def epilogue(pp, y, ctx):
    """Host: leftover probs, softmax, segment pooling, rank-1 classifier.

    pp: (N_CORES*kcap,) compact probs from device; y: (B, L) projection."""
    attn = ctx["attn"]
    mlm = ctx["mlm"]
    Wc, bc, scale = ctx["Wc"], ctx["bc"], ctx["scale"]
    pad_b, pad_t, n_dev = ctx["pad_b"], ctx["pad_t"], ctx["n_dev"]

    probs = np.zeros((B, L), np.float32)
    probs[pad_b[:n_dev], pad_t[:n_dev]] = pp[:n_dev]
    if n_dev < len(pad_b):  # leftover padded tokens, exact fp32 on host
        lb, lt = pad_b[n_dev:], pad_t[n_dev:]
        Xl = ctx["X"][lb, lt]                       # (n, H)
        kl = np.tanh(Xl @ ctx["Wh"].T + ctx["bh"])  # (n, A)
        probs[lb, lt] = (kl @ ctx["q"]) * scale

    maskmul = ((1.0 - attn.astype(np.float32)) * -1000.0)
    z = probs * maskmul
    z -= z.max(axis=1, keepdims=True)
    e = np.exp(z)
    att = e / e.sum(axis=1, keepdims=True)          # (B, L)

    idx = np.arange(L)
    marker = np.where(mlm > 0, idx[None, :], L)
    starts = np.sort(marker, axis=1)[:, :C]
    end_idx = attn.sum(axis=1)
    bounds = np.concatenate([starts[:, 1:] - 1, (end_idx - 1)[:, None]], axis=1)
    seg = ((idx[None, None, :] >= starts[:, :, None] + 1)
           & (idx[None, None, :] < bounds[:, :, None])).astype(np.float32)
    S_att = np.einsum("bcl,bl->bc", seg, att)
    Sy = np.einsum("bcl,bl->bc", seg, y)
    Wsum = Wc.sum(dtype=np.float32)
    return (S_att * Wsum + Sy + bc).astype(np.float32)[:, :, None]


_prog_cache = {}


def kernel(**inputs) -> np.ndarray:
    in_maps, ctx = prep_inputs(inputs)
    bias_free = not np.any(np.asarray(inputs["b_hidden"]))
    key = (ctx["kcap"], bias_free)
    if key not in _prog_cache:
        _prog_cache[key] = build_program(kcap=ctx["kcap"],
                                         bias_free=bias_free)
    nc = _prog_cache[key]
    res = run_bass_kernel_spmd(nc, in_maps, core_ids=list(range(N_CORES)))
    pp = np.concatenate(
        [res.results[c]["pp"][0] for c in range(N_CORES)])
    y = np.concatenate(
        [res.results[c]["y4"].sum(axis=0, dtype=np.float32).reshape(RPC, L)
         for c in range(N_CORES)])
    return epilogue(pp, y, ctx)


# revision 19
# speedup vs baseline: 5.5727x; 3.5767x over previous
"""Trainium2 Bass kernel for nn_ClassifyMLPHeadForKCRWithConcatChoices.

Math (B=16, L=2048, H=A=1024, C=5):
  keys  = tanh(X @ Wh^T + bh)                    (B,L,A)
  probs = keys @ (q / sqrt(A*var(q)))            (B,L)
  z     = probs * (-1000 * (1 - attn))           (B,L)
  att   = softmax_L(z)                           (B,L)
  vals  = att[...,None] + X                      (B,L,H)
  ctx   = einsum('bcl,blh->bch', seg, vals)
  logit = ctx @ Wc^T + bc                        (B,C,1)

Two structural facts make most of the FLOPs removable:

1. The softmax logits are ``probs * mask`` with mask = -1000*(1-attn), so
   z == 0 exactly wherever attn == 1.  probs (and hence the 68.7-GFLOP keys
   matmul) is only needed at PADDED tokens -- <= 511 per row, ~4.1K of 32.8K
   tokens total.  The device computes keys/probs only for a compacted,
   load-balanced gather of the padded tokens (fp8 DoubleRow matmul).
2. att broadcasts over H and the classifier is rank-1:
     logit[b,c] = (seg.att)[b,c]*sum(Wc) + (seg.y)[b,c] + bc,   y = X @ Wc
   so besides compact probs the device only needs the per-token projection
   y = X@Wc (bf16; its precision reaches the output).  y's matmul is rank-1
   (1 of 128 output partitions), so it is issued as 4 concurrent col-tiled
   matmuls (tile_position=(0,32j)) -- ~4x fewer PE cycles than sequential.

The tiny remainder (softmax over B*L scalars, segment pooling, any padded
tokens beyond device capacity) runs on the host during unsharding.

Sharding: y is data-parallel over batch (2 rows/core); compact padded tokens
are split evenly across the 8 cores regardless of row (probs is per-token).
"""

import sys

if '/opt/trn_rl_repo' not in sys.path:
    sys.path.insert(0, '/opt/trn_rl_repo')

import numpy as np
import ml_dtypes

import concourse.bass as bass  # noqa: F401  (bass must import before bacc)
import concourse.mybir as mybir
import concourse.tile as tile
from concourse import bacc
from concourse.bass_utils import run_bass_kernel_spmd

B, L, H, A, C = 16, 2048, 1024, 1024, 5
N_CORES = 8
RPC = B // N_CORES          # batch rows per core
NTOK = RPC * L              # tokens per core (y path)
P = 128
HB, AB = H // P, A // P     # contraction / output blocks
CH = 512                    # token chunk (one PSUM bank)
NCH = NTOK // CH
KCAP = 512                  # compact padded-token capacity per core (default)
YCAP = 3584                 # compact y-token capacity per core (default)
LEFTOVER_BUDGET = 256       # padded tokens beyond capacity handled on host

BF16 = mybir.dt.bfloat16
FP32 = mybir.dt.float32
FP8 = mybir.dt.float8e4
NP_FP8 = mybir.dt.np(FP8)


def build_program(repeat: int = 1, n_cores: int = N_CORES,
                  kcap: int = KCAP, ycap: int = YCAP, bias_free: bool = True,
                  pk_bufs: int = 2):
    """Compact keys/probs (fp8 DoubleRow over kcap gathered padded tokens)
    + col-tiled rank-1 classifier projection y over ycap gathered
    attended-or-segment tokens."""
    assert kcap % CH == 0 and ycap % CH == 0
    kch = kcap // CH
    ych = ycap // CH
    nc = bacc.Bacc("TRN2", target_bir_lowering=False, debug=False,
                   num_devices=n_cores)
    xt_d = nc.dram_tensor("xt", [HB, P, ycap], BF16, kind="ExternalInput")
    xc8_d = nc.dram_tensor("xc8", [P, HB * kcap], FP8, kind="ExternalInput")
    wht8_d = nc.dram_tensor("wht8", [P, HB * A], FP8, kind="ExternalInput")
    # q padded to 16B per a-block: dual-fp8 LDWEIGHTS requires the weight
    # AP's block step to be a multiple of 16 bytes
    qs_d = nc.dram_tensor("qs", [P, AB * 16], FP8, kind="ExternalInput")
    wc_d = nc.dram_tensor("wc", [P, HB], BF16, kind="ExternalInput")
    bh_d = nc.dram_tensor("bh", [P, AB], FP32, kind="ExternalInput")
    pp_d = nc.dram_tensor("pp", [1, kcap], FP32, kind="ExternalOutput")
    y4_d = nc.dram_tensor("y4", [4, ycap], FP32, kind="ExternalOutput")

    with tile.TileContext(nc) as tc:
        with (
            tc.tile_pool(name="const", bufs=1) as const,
            tc.tile_pool(name="xpool", bufs=1) as xpool,
            tc.tile_pool(name="keys", bufs=4) as keys,
            tc.tile_pool(name="vecs", bufs=1) as vecs,
            tc.tile_pool(name="ps_k", bufs=pk_bufs, space="PSUM") as ps_k,
            tc.tile_pool(name="ps_s", bufs=2, space="PSUM") as ps_s,
            tc.tile_pool(name="ps_y", bufs=2, space="PSUM") as ps_y,
        ):
            wht8_sb = const.tile([P, HB, A], FP8)
            nc.sync.dma_start(
                wht8_sb[:], wht8_d.ap().rearrange("p (h a) -> p h a", h=HB))
            qs_sb = const.tile([P, AB, 16], FP8)
            nc.sync.dma_start(
                qs_sb[:], qs_d.ap().rearrange("p (a s) -> p a s", a=AB))
            wc_sb = const.tile([P, HB], BF16)
            nc.sync.dma_start(wc_sb[:], wc_d.ap())
            bh_sb = const.tile([P, AB], FP32)
            nc.sync.dma_start(bh_sb[:], bh_d.ap())
            xc8_sb = const.tile([P, HB, kcap], FP8)
            nc.sync.dma_start(
                xc8_sb[:], xc8_d.ap().rearrange("p (h t) -> p h t", h=HB))
            # prefetch the exp_and_others ACT table set (covers Tanh)
            # during the input DMA window instead of at first real tanh
            warm = const.tile([1, 1], FP32)
            nc.scalar.activation(warm[:], bh_sb[:1, 0:1],
                                 mybir.ActivationFunctionType.Tanh)

            # X^T staged per (hb, chunk) for the y projection
            xt_sb = {}
            for ch in range(ych):
                for hb in range(HB):
                    t = xpool.tile([P, CH], BF16, tag=f"x{hb}_{ch}")
                    nc.sync.dma_start(
                        t[:], xt_d.ap()[hb, :, ch * CH:(ch + 1) * CH])
                    xt_sb[hb, ch] = t

            NP = AB // 2  # a-block pairs per chunk
            for _ in range(repeat):
                pp_sb = vecs.tile([1, kcap], FP32, tag="pp")
                # y partials staged [128, ych, CH]; only partitions
                # {0,32,64,96} are meaningful (col-tile outputs) -- the
                # final DMA gathers them with a partition-strided AP
                ys_sb = vecs.tile([P, ych, CH], FP32, tag="ys")

                # --- compact keys + probs over gathered padded tokens.
                # probs MMs are emitted 2 a-block-pairs behind their tanh
                # (and the last ones get interleaved into the y loop) so
                # the PE never stalls waiting on the scalar engine. ---
                pending = []  # (kc, abp, ks2, pprobs) probs MMs not emitted

                def emit_probs():
                    kc, abp, ks2, pprobs = pending.pop(0)
                    nc.tensor.matmul(
                        pprobs[:],
                        lhsT=qs_sb[:, 2 * abp:2 * abp + 2, 0:1],
                        rhs=ks2[:],
                        start=(abp == 0), stop=(abp == NP - 1),
                        perf_mode=mybir.MatmulPerfMode.DoubleRow)
                    if abp == NP - 1:  # chunk complete -> evacuate
                        nc.vector.tensor_copy(
                            pp_sb[:, kc * CH:(kc + 1) * CH], pprobs[:])

                for kc in range(kch):
                    csl = slice(kc * CH, (kc + 1) * CH)
                    pprobs = ps_s.tile([1, CH], FP32, tag="pprobs")
                    for abp in range(NP):
                        pk2 = ps_k.tile([P, 2, CH], FP32, tag="pk2")
                        for j in range(2):
                            ab = 2 * abp + j
                            for hbp in range(HB // 2):
                                nc.tensor.matmul(
                                    pk2[:, j, :],
                                    lhsT=wht8_sb[:, 2 * hbp:2 * hbp + 2,
                                                 ab * P:(ab + 1) * P],
                                    rhs=xc8_sb[:, 2 * hbp:2 * hbp + 2, csl],
                                    start=(hbp == 0),
                                    stop=(hbp == HB // 2 - 1),
                                    perf_mode=mybir.MatmulPerfMode.DoubleRow,
                                )
                        ks2 = keys.tile([P, 2, CH], FP8, tag="ks2")
                        if bias_free:
                            nc.scalar.activation(
                                ks2[:], pk2[:],
                                mybir.ActivationFunctionType.Tanh)
                        else:
                            for j in range(2):
                                nc.scalar.activation(
                                    ks2[:, j, :], pk2[:, j, :],
                                    mybir.ActivationFunctionType.Tanh,
                                    bias=bh_sb[:, 2 * abp + j:
                                               2 * abp + j + 1], scale=1.0)
                        pending.append((kc, abp, ks2, pprobs))
                        while len(pending) > 2:
                            emit_probs()
                    if kc < kch - 1:  # keep only the last chunk lagging
                        while pending:
                            emit_probs()

                # --- y = X @ Wc, 4 concurrent col-tiled rank-1 matmuls ---
                for ch in range(ych):
                    py = ps_y.tile([P, CH], FP32, tag="py")
                    for r in range(2):
                        for j in range(4):
                            hb = 4 * r + j
                            nc.tensor.matmul(
                                py[32 * j:32 * j + 1, :],
                                lhsT=wc_sb[:, hb:hb + 1],
                                rhs=xt_sb[hb, ch][:],
                                start=(r == 0), stop=(r == 1),
                                tile_position=(0, 32 * j),
                            )
                    if pending:
                        emit_probs()
                    nc.vector.tensor_copy(ys_sb[:, ch, :], py[:])
                while pending:
                    emit_probs()
                nc.sync.dma_start(pp_d.ap()[:], pp_sb[:])
                nc.sync.dma_start(
                    y4_d.ap().rearrange("f (c t) -> f c t", c=ych),
                    ys_sb[0:97:32, :, :])

    nc.compile()
    return nc


def prep_inputs(inputs):
    """Full inputs -> (per-core in_maps, host epilogue context)."""
    X = np.ascontiguousarray(np.asarray(inputs["input"], dtype=np.float32))
    attn = np.asarray(inputs["attention_mask"])
    mlm = np.asarray(inputs["mlm_mask"])
    Wh = np.asarray(inputs["W_hidden"], dtype=np.float32)
    bh = np.asarray(inputs["b_hidden"], dtype=np.float32)
    q = np.asarray(inputs["query"], dtype=np.float32)[:, 0]
    Wc = np.asarray(inputs["W_cls"], dtype=np.float32)[0]
    bc = float(np.asarray(inputs["b_cls"], dtype=np.float32)[0])

    qvar = np.var(q.astype(np.float64), ddof=1)
    scale = 1.0 / np.sqrt(A * qvar)

    qs = np.zeros((P, AB, 16), NP_FP8)
    qs[:, :, 0] = (q * scale).reshape(AB, P).T.astype(NP_FP8)
    qs = qs.reshape(P, AB * 16)
    wc = np.ascontiguousarray(Wc.reshape(HB, P).T).astype(ml_dtypes.bfloat16)
    bh_a = np.ascontiguousarray(bh.reshape(AB, P).T).astype(np.float32)
    WhT = np.ascontiguousarray(Wh.T)  # (H, A)
    # wht8[p, hb*A + a] = WhT[hb*128+p, a]
    wht8 = np.ascontiguousarray(
        WhT.reshape(HB, P, A).transpose(1, 0, 2).reshape(P, HB * A)
    ).astype(NP_FP8)

    # --- compact gather of padded tokens, load-balanced across cores ---
    pad_b, pad_t = np.nonzero(attn == 0)
    t_pad = len(pad_b)
    kcap = CH * max(1, int(np.ceil(
        max(t_pad - LEFTOVER_BUDGET, 1) / (N_CORES * CH))))
    n_dev = min(t_pad, N_CORES * kcap)
    flat_idx = (pad_b * L + pad_t)[:n_dev]
    # pad the slot array with token 0 (its probs output is ignored)
    slots = np.zeros(N_CORES * kcap, np.int64)
    slots[:n_dev] = flat_idx

    # --- compact gather of tokens needing y = X@Wc: attended or in a
    # segment (segment membership derives from the masks alone) ---
    seg = _seg_mask(attn, mlm)
    need_y = (attn > 0) | seg.any(axis=1)
    yb, yt = np.nonzero(need_y)
    t_y = len(yb)
    ycap = CH * max(1, min(NTOK // CH,
                           int(np.ceil(t_y / (N_CORES * CH)))))
    n_ydev = min(t_y, N_CORES * ycap)
    yslots = np.zeros(N_CORES * ycap, np.int64)
    yslots[:n_ydev] = (yb * L + yt)[:n_ydev]

    Xf = X.reshape(B * L, H)
    in_maps = []
    for c in range(N_CORES):
        ycols = yslots[c * ycap:(c + 1) * ycap]
        # xt[hb, p, t] = X[ycols[t], hb*128+p]
        xt_c = np.ascontiguousarray(
            Xf[ycols].T.reshape(HB, P, ycap)).astype(ml_dtypes.bfloat16)
        cols = slots[c * kcap:(c + 1) * kcap]
        # xc8[p, hb*kcap + t] = X[cols[t], hb*128+p]
        xc = Xf[cols].T.reshape(HB, P, kcap)  # (hb, p, t)
        xc8 = np.ascontiguousarray(
            xc.transpose(1, 0, 2).reshape(P, HB * kcap)).astype(NP_FP8)
        m = dict(
            xt=xt_c, xc8=xc8, wht8=wht8, qs=qs, wc=wc, bh=bh_a,
        )
        in_maps.append(m)
    host_ctx = dict(attn=attn, mlm=mlm, Wc=Wc, bc=bc, scale=scale,
                    Wh=Wh, bh=bh, q=q, X=X, kcap=kcap, n_dev=n_dev,
                    pad_b=pad_b, pad_t=pad_t, seg=seg,
                    ycap=ycap, n_ydev=n_ydev, yb=yb, yt=yt)
    return in_maps, host_ctx


def _seg_mask(attn, mlm):
    """(B, C, L) segment mask, exactly as the reference builds it."""
    idx = np.arange(L)
    marker = np.where(mlm > 0, idx[None, :], L)
    starts = np.sort(marker, axis=1)[:, :C]
    end_idx = attn.sum(axis=1)
    bounds = np.concatenate([starts[:, 1:] - 1, (end_idx - 1)[:, None]],
                            axis=1)
    return ((idx[None, None, :] >= starts[:, :, None] + 1)
            & (idx[None, None, :] < bounds[:, :, None]))


def epilogue(pp, y, ctx):
    """Host: leftover probs, softmax, segment pooling, rank-1 classifier.

    pp: (N_CORES*kcap,) compact probs from device; y: (B, L) projection."""
    attn = ctx["attn"]
    Wc, bc, scale = ctx["Wc"], ctx["bc"], ctx["scale"]
    pad_b, pad_t, n_dev = ctx["pad_b"], ctx["pad_t"], ctx["n_dev"]

    probs = np.zeros((B, L), np.float32)
    probs[pad_b[:n_dev], pad_t[:n_dev]] = pp[:n_dev]
    if n_dev < len(pad_b):  # leftover padded tokens, exact fp32 on host
        lb, lt = pad_b[n_dev:], pad_t[n_dev:]
        Xl = ctx["X"][lb, lt]                       # (n, H)
        kl = np.tanh(Xl @ ctx["Wh"].T + ctx["bh"])  # (n, A)
        probs[lb, lt] = (kl @ ctx["q"]) * scale

    maskmul = ((1.0 - attn.astype(np.float32)) * -1000.0)
    z = probs * maskmul
    z -= z.max(axis=1, keepdims=True)
    e = np.exp(z)
    att = e / e.sum(axis=1, keepdims=True)          # (B, L)

    seg = ctx["seg"].astype(np.float32)
    S_att = np.einsum("bcl,bl->bc", seg, att)
    Sy = np.einsum("bcl,bl->bc", seg, y)
    Wsum = Wc.sum(dtype=np.float32)
    return (S_att * Wsum + Sy + bc).astype(np.float32)[:, :, None]


_prog_cache = {}


def kernel(**inputs) -> np.ndarray:
    in_maps, ctx = prep_inputs(inputs)
    bias_free = not np.any(np.asarray(inputs["b_hidden"]))
    key = (ctx["kcap"], ctx["ycap"], bias_free)
    if key not in _prog_cache:
        _prog_cache[key] = build_program(kcap=ctx["kcap"],
                                         ycap=ctx["ycap"],
                                         bias_free=bias_free)
    nc = _prog_cache[key]
    res = run_bass_kernel_spmd(nc, in_maps, core_ids=list(range(N_CORES)))
    pp = np.concatenate(
        [res.results[c]["pp"][0] for c in range(N_CORES)])
    yflat = np.concatenate(
        [res.results[c]["y4"].sum(axis=0, dtype=np.float32)
         for c in range(N_CORES)])
    n_ydev, yb, yt = ctx["n_ydev"], ctx["yb"], ctx["yt"]
    y = np.zeros((B, L), np.float32)
    y[yb[:n_ydev], yt[:n_ydev]] = yflat[:n_ydev]
    if n_ydev < len(yb):  # y leftover beyond device capacity, on host
        lb, lt = yb[n_ydev:], yt[n_ydev:]
        y[lb, lt] = (ctx["X"][lb, lt] @ ctx["Wc"]).astype(np.float32)
    return epilogue(pp, y, ctx)


# revision 23
# speedup vs baseline: 7.2822x; 1.3068x over previous
"""Trainium2 Bass kernel for nn_ClassifyMLPHeadForKCRWithConcatChoices.

Math (B=16, L=2048, H=A=1024, C=5):
  keys  = tanh(X @ Wh^T + bh)                    (B,L,A)
  probs = keys @ (q / sqrt(A*var(q)))            (B,L)
  z     = probs * (-1000 * (1 - attn))           (B,L)
  att   = softmax_L(z)                           (B,L)
  vals  = att[...,None] + X                      (B,L,H)
  ctx   = einsum('bcl,blh->bch', seg, vals)
  logit = ctx @ Wc^T + bc                        (B,C,1)

Two structural facts make most of the FLOPs removable:

1. The softmax logits are ``probs * mask`` with mask = -1000*(1-attn), so
   z == 0 exactly wherever attn == 1.  probs (and hence the 68.7-GFLOP keys
   matmul) is only needed at PADDED tokens -- <= 511 per row, ~4.1K of 32.8K
   tokens total.  The device computes keys/probs only for a compacted,
   load-balanced gather of the padded tokens (fp8 DoubleRow matmul).
2. att broadcasts over H and the classifier is rank-1:
     logit[b,c] = (seg.att)[b,c]*sum(Wc) + (seg.y)[b,c] + bc,   y = X @ Wc
   so besides compact probs the device only needs the per-token projection
   y = X@Wc (bf16; its precision reaches the output).  y's matmul is rank-1
   (1 of 128 output partitions), so it is issued as 4 concurrent col-tiled
   matmuls (tile_position=(0,32j)) -- ~4x fewer PE cycles than sequential.

The tiny remainder (softmax over B*L scalars, segment pooling, any padded
tokens beyond device capacity) runs on the host during unsharding.

Sharding: both gathers are per-token, so the compact padded tokens (keys)
and the attended-or-segment tokens (y) are each split evenly across the 8
cores regardless of batch row; weights are replicated.
"""

import sys

if '/opt/trn_rl_repo' not in sys.path:
    sys.path.insert(0, '/opt/trn_rl_repo')

import numpy as np
import ml_dtypes

import concourse.bass as bass  # noqa: F401  (bass must import before bacc)
import concourse.mybir as mybir
import concourse.tile as tile
from concourse import bacc
from concourse.bass_utils import run_bass_kernel_spmd

B, L, H, A, C = 16, 2048, 1024, 1024, 5
N_CORES = 8
RPC = B // N_CORES          # batch rows per core
NTOK = RPC * L              # tokens per core (y path)
P = 128
HB, AB = H // P, A // P     # contraction / output blocks
CH = 512                    # token chunk (one PSUM bank)
NCH = NTOK // CH
KCAP = 512                  # compact padded-token capacity per core (default)
YCAP = 3584                 # compact y-token capacity per core (default)
LEFTOVER_BUDGET = 256       # padded tokens beyond capacity handled on host

BF16 = mybir.dt.bfloat16
FP32 = mybir.dt.float32
FP8 = mybir.dt.float8e4
NP_FP8 = mybir.dt.np(FP8)


def build_program(repeat: int = 1, n_cores: int = N_CORES,
                  kcap: int = KCAP, ycap: int = YCAP, bias_free: bool = True,
                  pk_bufs: int = 2):
    """Compact keys/probs (fp8 DoubleRow over kcap gathered padded tokens)
    + col-tiled rank-1 classifier projection y over ycap gathered
    attended-or-segment tokens."""
    assert kcap % CH == 0 and ycap % CH == 0
    kch = kcap // CH
    ych = ycap // CH
    nc = bacc.Bacc("TRN2", target_bir_lowering=False, debug=False,
                   num_devices=n_cores)
    xt_d = nc.dram_tensor("xt", [HB, P, ycap], BF16, kind="ExternalInput")
    xc8_d = nc.dram_tensor("xc8", [P, HB * kcap], FP8, kind="ExternalInput")
    wht8_d = nc.dram_tensor("wht8", [P, HB * A], FP8, kind="ExternalInput")
    # q padded to 16B per a-block: dual-fp8 LDWEIGHTS requires the weight
    # AP's block step to be a multiple of 16 bytes
    qs_d = nc.dram_tensor("qs", [P, AB * 16], FP8, kind="ExternalInput")
    wc_d = nc.dram_tensor("wc", [P, HB], BF16, kind="ExternalInput")
    bh_d = nc.dram_tensor("bh", [P, AB], FP32, kind="ExternalInput")
    pp_d = nc.dram_tensor("pp", [1, kcap], FP32, kind="ExternalOutput")
    y4_d = nc.dram_tensor("y4", [4, ycap], FP32, kind="ExternalOutput")

    with tile.TileContext(nc) as tc:
        with (
            tc.tile_pool(name="const", bufs=1) as const,
            tc.tile_pool(name="xpool", bufs=1) as xpool,
            tc.tile_pool(name="keys", bufs=4) as keys,
            tc.tile_pool(name="vecs", bufs=1) as vecs,
            tc.tile_pool(name="ps_k", bufs=pk_bufs, space="PSUM") as ps_k,
            tc.tile_pool(name="ps_s", bufs=2, space="PSUM") as ps_s,
            tc.tile_pool(name="ps_y", bufs=2, space="PSUM") as ps_y,
        ):
            wht8_sb = const.tile([P, HB, A], FP8)
            nc.sync.dma_start(
                wht8_sb[:], wht8_d.ap().rearrange("p (h a) -> p h a", h=HB))
            qs_sb = const.tile([P, AB, 16], FP8)
            nc.sync.dma_start(
                qs_sb[:], qs_d.ap().rearrange("p (a s) -> p a s", a=AB))
            wc_sb = const.tile([P, HB], BF16)
            nc.sync.dma_start(wc_sb[:], wc_d.ap())
            bh_sb = const.tile([P, AB], FP32)
            nc.sync.dma_start(bh_sb[:], bh_d.ap())
            xc8_sb = const.tile([P, HB, kcap], FP8)
            nc.sync.dma_start(
                xc8_sb[:], xc8_d.ap().rearrange("p (h t) -> p h t", h=HB))
            # prefetch the exp_and_others ACT table set (covers Tanh)
            # during the input DMA window instead of at first real tanh
            warm = const.tile([1, 1], FP32)
            nc.scalar.activation(warm[:], bh_sb[:1, 0:1],
                                 mybir.ActivationFunctionType.Tanh)

            # X^T staged per (hb, chunk) for the y projection
            xt_sb = {}
            for ch in range(ych):
                for hb in range(HB):
                    t = xpool.tile([P, CH], BF16, tag=f"x{hb}_{ch}")
                    nc.sync.dma_start(
                        t[:], xt_d.ap()[hb, :, ch * CH:(ch + 1) * CH])
                    xt_sb[hb, ch] = t

            NP = AB // 2  # a-block pairs per chunk
            for _ in range(repeat):
                pp_sb = vecs.tile([1, kcap], FP32, tag="pp")
                # y partials staged [128, ych, CH]; only partitions
                # {0,32,64,96} are meaningful (col-tile outputs) -- the
                # final DMA gathers them with a partition-strided AP
                ys_sb = vecs.tile([P, ych, CH], FP32, tag="ys")

                # --- compact keys + probs over gathered padded tokens ---
                for kc in range(kch):
                    csl = slice(kc * CH, (kc + 1) * CH)
                    pprobs = ps_s.tile([1, CH], FP32, tag="pprobs")
                    for abp in range(NP):
                        pk2 = ps_k.tile([P, 2, CH], FP32, tag="pk2")
                        for j in range(2):
                            ab = 2 * abp + j
                            for hbp in range(HB // 2):
                                nc.tensor.matmul(
                                    pk2[:, j, :],
                                    lhsT=wht8_sb[:, 2 * hbp:2 * hbp + 2,
                                                 ab * P:(ab + 1) * P],
                                    rhs=xc8_sb[:, 2 * hbp:2 * hbp + 2, csl],
                                    start=(hbp == 0),
                                    stop=(hbp == HB // 2 - 1),
                                    perf_mode=mybir.MatmulPerfMode.DoubleRow,
                                )
                        ks2 = keys.tile([P, 2, CH], FP8, tag="ks2")
                        if bias_free:
                            nc.scalar.activation(
                                ks2[:], pk2[:],
                                mybir.ActivationFunctionType.Tanh)
                        else:
                            for j in range(2):
                                nc.scalar.activation(
                                    ks2[:, j, :], pk2[:, j, :],
                                    mybir.ActivationFunctionType.Tanh,
                                    bias=bh_sb[:, 2 * abp + j:
                                               2 * abp + j + 1], scale=1.0)
                        nc.tensor.matmul(
                            pprobs[:],
                            lhsT=qs_sb[:, 2 * abp:2 * abp + 2, 0:1],
                            rhs=ks2[:],
                            start=(abp == 0), stop=(abp == NP - 1),
                            perf_mode=mybir.MatmulPerfMode.DoubleRow)
                    nc.vector.tensor_copy(pp_sb[:, csl], pprobs[:])

                # --- y = X @ Wc, 4 concurrent col-tiled rank-1 matmuls ---
                for ch in range(ych):
                    py = ps_y.tile([P, CH], FP32, tag="py")
                    for r in range(2):
                        for j in range(4):
                            hb = 4 * r + j
                            nc.tensor.matmul(
                                py[32 * j:32 * j + 1, :],
                                lhsT=wc_sb[:, hb:hb + 1],
                                rhs=xt_sb[hb, ch][:],
                                start=(r == 0), stop=(r == 1),
                                tile_position=(0, 32 * j),
                            )
                    nc.vector.tensor_copy(ys_sb[:, ch, :], py[:])
                nc.sync.dma_start(pp_d.ap()[:], pp_sb[:])
                nc.sync.dma_start(
                    y4_d.ap().rearrange("f (c t) -> f c t", c=ych),
                    ys_sb[0:97:32, :, :])

    nc.compile()
    return nc


def prep_inputs(inputs):
    """Full inputs -> (per-core in_maps, host epilogue context)."""
    X = np.ascontiguousarray(np.asarray(inputs["input"], dtype=np.float32))
    attn = np.asarray(inputs["attention_mask"])
    mlm = np.asarray(inputs["mlm_mask"])
    Wh = np.asarray(inputs["W_hidden"], dtype=np.float32)
    bh = np.asarray(inputs["b_hidden"], dtype=np.float32)
    q = np.asarray(inputs["query"], dtype=np.float32)[:, 0]
    Wc = np.asarray(inputs["W_cls"], dtype=np.float32)[0]
    bc = float(np.asarray(inputs["b_cls"], dtype=np.float32)[0])

    qvar = np.var(q.astype(np.float64), ddof=1)
    scale = 1.0 / np.sqrt(A * qvar)

    qs = np.zeros((P, AB, 16), NP_FP8)
    qs[:, :, 0] = (q * scale).reshape(AB, P).T.astype(NP_FP8)
    qs = qs.reshape(P, AB * 16)
    wc = np.ascontiguousarray(Wc.reshape(HB, P).T).astype(ml_dtypes.bfloat16)
    bh_a = np.ascontiguousarray(bh.reshape(AB, P).T).astype(np.float32)
    WhT = np.ascontiguousarray(Wh.T)  # (H, A)
    # wht8[p, hb*A + a] = WhT[hb*128+p, a]
    wht8 = np.ascontiguousarray(
        WhT.reshape(HB, P, A).transpose(1, 0, 2).reshape(P, HB * A)
    ).astype(NP_FP8)

    # --- compact gather of padded tokens, load-balanced across cores ---
    pad_b, pad_t = np.nonzero(attn == 0)
    t_pad = len(pad_b)
    kcap = CH * max(1, int(np.ceil(
        max(t_pad - LEFTOVER_BUDGET, 1) / (N_CORES * CH))))
    n_dev = min(t_pad, N_CORES * kcap)
    flat_idx = (pad_b * L + pad_t)[:n_dev]
    # pad the slot array with token 0 (its probs output is ignored)
    slots = np.zeros(N_CORES * kcap, np.int64)
    slots[:n_dev] = flat_idx

    # --- compact gather of tokens needing y = X@Wc: attended or in a
    # segment (segment membership derives from the masks alone) ---
    seg = _seg_mask(attn, mlm)
    need_y = (attn > 0) | seg.any(axis=1)
    yb, yt = np.nonzero(need_y)
    t_y = len(yb)
    ycap = CH * max(1, min(NTOK // CH,
                           int(np.ceil(t_y / (N_CORES * CH)))))
    n_ydev = min(t_y, N_CORES * ycap)
    yslots = np.zeros(N_CORES * ycap, np.int64)
    yslots[:n_ydev] = (yb * L + yt)[:n_ydev]

    Xf = X.reshape(B * L, H)
    in_maps = []
    for c in range(N_CORES):
        ycols = yslots[c * ycap:(c + 1) * ycap]
        # xt[hb, p, t] = X[ycols[t], hb*128+p]
        xt_c = np.ascontiguousarray(
            Xf[ycols].T.reshape(HB, P, ycap)).astype(ml_dtypes.bfloat16)
        cols = slots[c * kcap:(c + 1) * kcap]
        # xc8[p, hb*kcap + t] = X[cols[t], hb*128+p]
        xc = Xf[cols].T.reshape(HB, P, kcap)  # (hb, p, t)
        xc8 = np.ascontiguousarray(
            xc.transpose(1, 0, 2).reshape(P, HB * kcap)).astype(NP_FP8)
        m = dict(
            xt=xt_c, xc8=xc8, wht8=wht8, qs=qs, wc=wc, bh=bh_a,
        )
        in_maps.append(m)
    host_ctx = dict(attn=attn, mlm=mlm, Wc=Wc, bc=bc, scale=scale,
                    Wh=Wh, bh=bh, q=q, X=X, kcap=kcap, n_dev=n_dev,
                    pad_b=pad_b, pad_t=pad_t, seg=seg,
                    ycap=ycap, n_ydev=n_ydev, yb=yb, yt=yt)
    return in_maps, host_ctx


def _seg_mask(attn, mlm):
    """(B, C, L) segment mask, exactly as the reference builds it."""
    idx = np.arange(L)
    marker = np.where(mlm > 0, idx[None, :], L)
    starts = np.sort(marker, axis=1)[:, :C]
    end_idx = attn.sum(axis=1)
    bounds = np.concatenate([starts[:, 1:] - 1, (end_idx - 1)[:, None]],
                            axis=1)
    return ((idx[None, None, :] >= starts[:, :, None] + 1)
            & (idx[None, None, :] < bounds[:, :, None]))


def epilogue(pp, y, ctx):
    """Host: leftover probs, softmax, segment pooling, rank-1 classifier.

    pp: (N_CORES*kcap,) compact probs from device; y: (B, L) projection."""
    attn = ctx["attn"]
    Wc, bc, scale = ctx["Wc"], ctx["bc"], ctx["scale"]
    pad_b, pad_t, n_dev = ctx["pad_b"], ctx["pad_t"], ctx["n_dev"]

    probs = np.zeros((B, L), np.float32)
    probs[pad_b[:n_dev], pad_t[:n_dev]] = pp[:n_dev]
    if n_dev < len(pad_b):  # leftover padded tokens, exact fp32 on host
        lb, lt = pad_b[n_dev:], pad_t[n_dev:]
        Xl = ctx["X"][lb, lt]                       # (n, H)
        kl = np.tanh(Xl @ ctx["Wh"].T + ctx["bh"])  # (n, A)
        probs[lb, lt] = (kl @ ctx["q"]) * scale

    maskmul = ((1.0 - attn.astype(np.float32)) * -1000.0)
    z = probs * maskmul
    z -= z.max(axis=1, keepdims=True)
    e = np.exp(z)
    att = e / e.sum(axis=1, keepdims=True)          # (B, L)

    seg = ctx["seg"].astype(np.float32)
    S_att = np.einsum("bcl,bl->bc", seg, att)
    Sy = np.einsum("bcl,bl->bc", seg, y)
    Wsum = Wc.sum(dtype=np.float32)
    return (S_att * Wsum + Sy + bc).astype(np.float32)[:, :, None]


_prog_cache = {}


def kernel(**inputs) -> np.ndarray:
    in_maps, ctx = prep_inputs(inputs)
    bias_free = not np.any(np.asarray(inputs["b_hidden"]))
    key = (ctx["kcap"], ctx["ycap"], bias_free)
    if key not in _prog_cache:
        _prog_cache[key] = build_program(kcap=ctx["kcap"],
                                         ycap=ctx["ycap"],
                                         bias_free=bias_free)
    nc = _prog_cache[key]
    res = run_bass_kernel_spmd(nc, in_maps, core_ids=list(range(N_CORES)))
    pp = np.concatenate(
        [res.results[c]["pp"][0] for c in range(N_CORES)])
    yflat = np.concatenate(
        [res.results[c]["y4"].sum(axis=0, dtype=np.float32)
         for c in range(N_CORES)])
    n_ydev, yb, yt = ctx["n_ydev"], ctx["yb"], ctx["yt"]
    y = np.zeros((B, L), np.float32)
    y[yb[:n_ydev], yt[:n_ydev]] = yflat[:n_ydev]
    if n_ydev < len(yb):  # y leftover beyond device capacity, on host
        lb, lt = yb[n_ydev:], yt[n_ydev:]
        y[lb, lt] = (ctx["X"][lb, lt] @ ctx["Wc"]).astype(np.float32)
    return epilogue(pp, y, ctx)


# revision 31
# speedup vs baseline: 7.8598x; 1.0793x over previous
"""Trainium2 Bass kernel for nn_ClassifyMLPHeadForKCRWithConcatChoices.

Math (B=16, L=2048, H=A=1024, C=5):
  keys  = tanh(X @ Wh^T + bh)                    (B,L,A)
  probs = keys @ (q / sqrt(A*var(q)))            (B,L)
  z     = probs * (-1000 * (1 - attn))           (B,L)
  att   = softmax_L(z)                           (B,L)
  vals  = att[...,None] + X                      (B,L,H)
  ctx   = einsum('bcl,blh->bch', seg, vals)
  logit = ctx @ Wc^T + bc                        (B,C,1)

Two structural facts make most of the FLOPs removable:

1. The softmax logits are ``probs * mask`` with mask = -1000*(1-attn), so
   z == 0 exactly wherever attn == 1.  probs (and hence the 68.7-GFLOP keys
   matmul) is only needed at PADDED tokens -- <= 511 per row, ~4.1K of 32.8K
   tokens total.  The device computes keys/probs only for a compacted,
   load-balanced gather of the padded tokens (fp8 DoubleRow matmul).
2. att broadcasts over H and the classifier is rank-1:
     logit[b,c] = (seg.att)[b,c]*sum(Wc) + (seg.y)[b,c] + bc,   y = X @ Wc
   so besides compact probs the device only needs the per-token projection
   y = X@Wc (bf16; its precision reaches the output).  y's matmul is rank-1
   (1 of 128 output partitions), so it is issued as 4 concurrent col-tiled
   matmuls (tile_position=(0,32j)) -- ~4x fewer PE cycles than sequential.

The tiny remainder (softmax over B*L scalars, segment pooling, any padded
tokens beyond device capacity) runs on the host during unsharding.

Sharding: both gathers are per-token, so the compact padded tokens (keys)
and the attended-or-segment tokens (y) are each split evenly across the 8
cores regardless of batch row; weights are replicated.
"""

import sys

if '/opt/trn_rl_repo' not in sys.path:
    sys.path.insert(0, '/opt/trn_rl_repo')

import numpy as np
import ml_dtypes

import concourse.bass as bass  # noqa: F401  (bass must import before bacc)
import concourse.mybir as mybir
import concourse.tile as tile
from concourse import bacc
from concourse.bass_utils import run_bass_kernel_spmd

B, L, H, A, C = 16, 2048, 1024, 1024, 5
N_CORES = 8
RPC = B // N_CORES          # batch rows per core
NTOK = RPC * L              # tokens per core (y path)
P = 128
HB, AB = H // P, A // P     # contraction / output blocks
CH = 512                    # token chunk (one PSUM bank)
NCH = NTOK // CH
KCAP = 512                  # compact padded-token capacity per core (default)
YCAP = 3584                 # compact y-token capacity per core (default)
LEFTOVER_BUDGET = 256       # padded tokens beyond capacity handled on host

BF16 = mybir.dt.bfloat16
FP32 = mybir.dt.float32
FP8 = mybir.dt.float8e4
NP_FP8 = mybir.dt.np(FP8)


def build_program(repeat: int = 1, n_cores: int = N_CORES,
                  kcap: int = KCAP, ycap: int = YCAP, bias_free: bool = True,
                  pk_bufs: int = 2, stages: str = "full"):
    """Compact keys/probs (fp8 DoubleRow over kcap gathered padded tokens)
    + col-tiled rank-1 classifier projection y over ycap gathered
    attended-or-segment tokens."""
    assert kcap % CH == 0 and ycap % CH == 0
    kch = kcap // CH
    ych = ycap // CH
    nc = bacc.Bacc("TRN2", target_bir_lowering=False, debug=False,
                   num_devices=n_cores)
    xt_d = nc.dram_tensor("xt", [HB, P, ycap], BF16, kind="ExternalInput")
    xc8_d = nc.dram_tensor("xc8", [P, HB * kcap], FP8, kind="ExternalInput")
    wht8_d = nc.dram_tensor("wht8", [P, HB * A], FP8, kind="ExternalInput")
    # q padded to 16B per a-block: dual-fp8 LDWEIGHTS requires the weight
    # AP's block step to be a multiple of 16 bytes
    qs_d = nc.dram_tensor("qs", [P, AB * 16], FP8, kind="ExternalInput")
    wc_d = nc.dram_tensor("wc", [P, HB], BF16, kind="ExternalInput")
    bh_d = nc.dram_tensor("bh", [P, AB], FP32, kind="ExternalInput")
    pp_d = nc.dram_tensor("pp", [1, kcap], FP32, kind="ExternalOutput")
    # y partials leave as bf16: their rounding (~0.2% on a term that is
    # itself bf16-limited) is invisible next to the 2e-2 gate, and it
    # halves the DVE evacuation time that would otherwise pace the y loop
    y4_d = nc.dram_tensor("y4", [4, ycap], BF16, kind="ExternalOutput")

    with tile.TileContext(nc) as tc:
        with (
            tc.tile_pool(name="const", bufs=1) as const,
            tc.tile_pool(name="xpool", bufs=1) as xpool,
            tc.tile_pool(name="keys", bufs=4) as keys,
            tc.tile_pool(name="vecs", bufs=1) as vecs,
            tc.tile_pool(name="ps_k", bufs=pk_bufs, space="PSUM") as ps_k,
            tc.tile_pool(name="ps_s", bufs=1, space="PSUM") as ps_s,
            tc.tile_pool(name="ps_y", bufs=3, space="PSUM") as ps_y,
        ):
            wht8_sb = const.tile([P, HB, A], FP8)
            nc.sync.dma_start(
                wht8_sb[:], wht8_d.ap().rearrange("p (h a) -> p h a", h=HB))
            qs_sb = const.tile([P, AB, 16], FP8)
            nc.sync.dma_start(
                qs_sb[:], qs_d.ap().rearrange("p (a s) -> p a s", a=AB))
            wc_sb = const.tile([P, HB], BF16)
            nc.sync.dma_start(wc_sb[:], wc_d.ap())
            bh_sb = const.tile([P, AB], FP32)
            nc.sync.dma_start(bh_sb[:], bh_d.ap())
            xc8_sb = const.tile([P, HB, kcap], FP8)
            nc.sync.dma_start(
                xc8_sb[:], xc8_d.ap().rearrange("p (h t) -> p h t", h=HB))
            # prefetch the exp_and_others ACT table set (covers Tanh)
            # during the input DMA window instead of at first real tanh
            warm = const.tile([1, 1], FP32)
            nc.scalar.activation(warm[:], bh_sb[:1, 0:1],
                                 mybir.ActivationFunctionType.Tanh)

            # X^T staged per (hb, chunk) for the y projection
            xt_sb = {}
            for ch in range(ych):
                for hb in range(HB):
                    t = xpool.tile([P, CH], BF16, tag=f"x{hb}_{ch}")
                    nc.sync.dma_start(
                        t[:], xt_d.ap()[hb, :, ch * CH:(ch + 1) * CH])
                    xt_sb[hb, ch] = t

            NP = AB // 2  # a-block pairs per chunk
            for _ in range(repeat):
                pp_sb = vecs.tile([1, kcap], FP32, tag="pp")
                # y partials staged [128, ych, CH]; only partitions
                # {0,32,64,96} are meaningful (col-tile outputs) -- the
                # final DMA gathers them with a partition-strided AP
                ys_sb = vecs.tile([P, ych, CH], BF16, tag="ys")

                # --- compact keys + probs over gathered padded tokens ---
                for kc in range(kch if stages in ("full", "keys") else 0):
                    csl = slice(kc * CH, (kc + 1) * CH)
                    pprobs = ps_s.tile([1, CH], FP32, tag="pprobs")
                    for abp in range(NP):
                        pk2 = ps_k.tile([P, 2, CH], FP32, tag="pk2")
                        for j in range(2):
                            ab = 2 * abp + j
                            for hbp in range(HB // 2):
                                nc.tensor.matmul(
                                    pk2[:, j, :],
                                    lhsT=wht8_sb[:, 2 * hbp:2 * hbp + 2,
                                                 ab * P:(ab + 1) * P],
                                    rhs=xc8_sb[:, 2 * hbp:2 * hbp + 2, csl],
                                    start=(hbp == 0),
                                    stop=(hbp == HB // 2 - 1),
                                    perf_mode=mybir.MatmulPerfMode.DoubleRow,
                                )
                        ks2 = keys.tile([P, 2, CH], FP8, tag="ks2")
                        if bias_free:
                            nc.scalar.activation(
                                ks2[:], pk2[:],
                                mybir.ActivationFunctionType.Tanh)
                        else:
                            for j in range(2):
                                nc.scalar.activation(
                                    ks2[:, j, :], pk2[:, j, :],
                                    mybir.ActivationFunctionType.Tanh,
                                    bias=bh_sb[:, 2 * abp + j:
                                               2 * abp + j + 1], scale=1.0)
                        nc.tensor.matmul(
                            pprobs[:],
                            lhsT=qs_sb[:, 2 * abp:2 * abp + 2, 0:1],
                            rhs=ks2[:],
                            start=(abp == 0), stop=(abp == NP - 1),
                            perf_mode=mybir.MatmulPerfMode.DoubleRow)
                    nc.vector.tensor_copy(pp_sb[:, csl], pprobs[:])

                # --- y = X @ Wc, 4 concurrent col-tiled rank-1 matmuls ---
                for ch in range(ych if stages in ("full", "y") else 0):
                    py = ps_y.tile([P, CH], FP32, tag="py")
                    for r in range(2):
                        for j in range(4):
                            hb = 4 * r + j
                            nc.tensor.matmul(
                                py[32 * j:32 * j + 1, :],
                                lhsT=wc_sb[:, hb:hb + 1],
                                rhs=xt_sb[hb, ch][:],
                                start=(r == 0), stop=(r == 1),
                                tile_position=(0, 32 * j),
                            )
                    nc.vector.tensor_copy(ys_sb[:, ch, :], py[:])
                if stages in ("full", "keys"):
                    nc.sync.dma_start(pp_d.ap()[:], pp_sb[:])
                if stages in ("full", "y"):
                    nc.sync.dma_start(
                        y4_d.ap().rearrange("f (c t) -> f c t", c=ych),
                        ys_sb[0:97:32, :, :])

    nc.compile()
    return nc


def prep_inputs(inputs):
    """Full inputs -> (per-core in_maps, host epilogue context)."""
    X = np.ascontiguousarray(np.asarray(inputs["input"], dtype=np.float32))
    attn = np.asarray(inputs["attention_mask"])
    mlm = np.asarray(inputs["mlm_mask"])
    Wh = np.asarray(inputs["W_hidden"], dtype=np.float32)
    bh = np.asarray(inputs["b_hidden"], dtype=np.float32)
    q = np.asarray(inputs["query"], dtype=np.float32)[:, 0]
    Wc = np.asarray(inputs["W_cls"], dtype=np.float32)[0]
    bc = float(np.asarray(inputs["b_cls"], dtype=np.float32)[0])

    qvar = np.var(q.astype(np.float64), ddof=1)
    scale = 1.0 / np.sqrt(A * qvar)

    qs = np.zeros((P, AB, 16), NP_FP8)
    qs[:, :, 0] = (q * scale).reshape(AB, P).T.astype(NP_FP8)
    qs = qs.reshape(P, AB * 16)
    wc = np.ascontiguousarray(Wc.reshape(HB, P).T).astype(ml_dtypes.bfloat16)
    bh_a = np.ascontiguousarray(bh.reshape(AB, P).T).astype(np.float32)
    WhT = np.ascontiguousarray(Wh.T)  # (H, A)
    # wht8[p, hb*A + a] = WhT[hb*128+p, a]
    wht8 = np.ascontiguousarray(
        WhT.reshape(HB, P, A).transpose(1, 0, 2).reshape(P, HB * A)
    ).astype(NP_FP8)

    # --- compact gather of padded tokens, load-balanced across cores ---
    pad_b, pad_t = np.nonzero(attn == 0)
    t_pad = len(pad_b)
    kcap = CH * max(1, int(np.ceil(
        max(t_pad - LEFTOVER_BUDGET, 1) / (N_CORES * CH))))
    n_dev = min(t_pad, N_CORES * kcap)
    flat_idx = (pad_b * L + pad_t)[:n_dev]
    # pad the slot array with token 0 (its probs output is ignored)
    slots = np.zeros(N_CORES * kcap, np.int64)
    slots[:n_dev] = flat_idx

    # --- compact gather of tokens needing y = X@Wc: attended or in a
    # segment (segment membership derives from the masks alone) ---
    seg = _seg_mask(attn, mlm)
    need_y = (attn > 0) | seg.any(axis=1)
    yb, yt = np.nonzero(need_y)
    t_y = len(yb)
    ycap = CH * max(1, min(NTOK // CH,
                           int(np.ceil(t_y / (N_CORES * CH)))))
    n_ydev = min(t_y, N_CORES * ycap)
    yslots = np.zeros(N_CORES * ycap, np.int64)
    yslots[:n_ydev] = (yb * L + yt)[:n_ydev]

    Xf = X.reshape(B * L, H)
    in_maps = []
    for c in range(N_CORES):
        ycols = yslots[c * ycap:(c + 1) * ycap]
        # xt[hb, p, t] = X[ycols[t], hb*128+p]
        xt_c = np.ascontiguousarray(
            Xf[ycols].T.reshape(HB, P, ycap)).astype(ml_dtypes.bfloat16)
        cols = slots[c * kcap:(c + 1) * kcap]
        # xc8[p, hb*kcap + t] = X[cols[t], hb*128+p]
        xc = Xf[cols].T.reshape(HB, P, kcap)  # (hb, p, t)
        xc8 = np.ascontiguousarray(
            xc.transpose(1, 0, 2).reshape(P, HB * kcap)).astype(NP_FP8)
        m = dict(
            xt=xt_c, xc8=xc8, wht8=wht8, qs=qs, wc=wc, bh=bh_a,
        )
        in_maps.append(m)
    host_ctx = dict(attn=attn, mlm=mlm, Wc=Wc, bc=bc, scale=scale,
                    Wh=Wh, bh=bh, q=q, X=X, kcap=kcap, n_dev=n_dev,
                    pad_b=pad_b, pad_t=pad_t, seg=seg,
                    ycap=ycap, n_ydev=n_ydev, yb=yb, yt=yt)
    return in_maps, host_ctx


def _seg_mask(attn, mlm):
    """(B, C, L) segment mask, exactly as the reference builds it."""
    idx = np.arange(L)
    marker = np.where(mlm > 0, idx[None, :], L)
    starts = np.sort(marker, axis=1)[:, :C]
    end_idx = attn.sum(axis=1)
    bounds = np.concatenate([starts[:, 1:] - 1, (end_idx - 1)[:, None]],
                            axis=1)
    return ((idx[None, None, :] >= starts[:, :, None] + 1)
            & (idx[None, None, :] < bounds[:, :, None]))


def epilogue(pp, y, ctx):
    """Host: leftover probs, softmax, segment pooling, rank-1 classifier.

    pp: (N_CORES*kcap,) compact probs from device; y: (B, L) projection."""
    attn = ctx["attn"]
    Wc, bc, scale = ctx["Wc"], ctx["bc"], ctx["scale"]
    pad_b, pad_t, n_dev = ctx["pad_b"], ctx["pad_t"], ctx["n_dev"]

    probs = np.zeros((B, L), np.float32)
    probs[pad_b[:n_dev], pad_t[:n_dev]] = pp[:n_dev]
    if n_dev < len(pad_b):  # leftover padded tokens, exact fp32 on host
        lb, lt = pad_b[n_dev:], pad_t[n_dev:]
        Xl = ctx["X"][lb, lt]                       # (n, H)
        kl = np.tanh(Xl @ ctx["Wh"].T + ctx["bh"])  # (n, A)
        probs[lb, lt] = (kl @ ctx["q"]) * scale

    maskmul = ((1.0 - attn.astype(np.float32)) * -1000.0)
    z = probs * maskmul
    z -= z.max(axis=1, keepdims=True)
    e = np.exp(z)
    att = e / e.sum(axis=1, keepdims=True)          # (B, L)

    seg = ctx["seg"].astype(np.float32)
    S_att = np.einsum("bcl,bl->bc", seg, att)
    Sy = np.einsum("bcl,bl->bc", seg, y)
    Wsum = Wc.sum(dtype=np.float32)
    return (S_att * Wsum + Sy + bc).astype(np.float32)[:, :, None]


_prog_cache = {}


def kernel(**inputs) -> np.ndarray:
    in_maps, ctx = prep_inputs(inputs)
    bias_free = not np.any(np.asarray(inputs["b_hidden"]))
    key = (ctx["kcap"], ctx["ycap"], bias_free)
    if key not in _prog_cache:
        _prog_cache[key] = build_program(kcap=ctx["kcap"],
                                         ycap=ctx["ycap"],
                                         bias_free=bias_free)
    nc = _prog_cache[key]
    res = run_bass_kernel_spmd(nc, in_maps, core_ids=list(range(N_CORES)))
    pp = np.concatenate(
        [res.results[c]["pp"][0] for c in range(N_CORES)])
    yflat = np.concatenate(
        [res.results[c]["y4"].astype(np.float32).sum(axis=0)
         for c in range(N_CORES)])
    n_ydev, yb, yt = ctx["n_ydev"], ctx["yb"], ctx["yt"]
    y = np.zeros((B, L), np.float32)
    y[yb[:n_ydev], yt[:n_ydev]] = yflat[:n_ydev]
    if n_ydev < len(yb):  # y leftover beyond device capacity, on host
        lb, lt = yb[n_ydev:], yt[n_ydev:]
        y[lb, lt] = (ctx["X"][lb, lt] @ ctx["Wc"]).astype(np.float32)
    return epilogue(pp, y, ctx)
